# revision 1
# baseline (speedup 1.0000x reference)
"""Trainium2 Bass kernel for the ESN (echo state network) forward scan.

  x_{t+1} = (1-a) x_t + a tanh(u_t + x_t @ W),  a = 0.5
  U = einsum('bit,in->tbn', Input, W_in);  out X[b,n,t] = x_{t+1}[b,n]

Sharding: data-parallel over batch (B=64 -> 8 cores x 8 batches).
W, W_in replicated; no collectives. Each core runs the full T=2000 scan
for its 8 batches and writes its [8, 1024, 2000] output slice.

Per-core per-step data flow (all on-chip, only X streamed out):
  PE:  z[8,1024](PSUM) = xT.T @ W (8 k-tile matmuls x 2 psum banks)
                        + inp_t.T @ W_in (u folded in as a K=16 matmul)
  DVE: z -> zT [128, 64] (32x32 block transposes, strided APs)
  ACT: hT = tanh(zT)
  DVE: s = xT + hT; xT' = 0.5 s; obuf[:, :, t] = 0.5 s   (x_{t+1})
Output chunks of Tc steps buffered in SBUF, DMA'd as [128, Tc]-contiguous
blocks into X[b, 128g:128g+128, t0:t0+Tc].
"""

import copy
import math
import os
import numpy as np

import concourse.bass as bass
import concourse.mybir as mybir
import concourse.tile as tile
from concourse.bass import ds
from concourse.bass_utils import run_bass_kernel_spmd

FP32 = mybir.dt.float32
FP16 = mybir.dt.float16

ALPHA = 0.5
N_CORES = 8
B, N_IN, T, N = 64, 16, 2000, 1024
TC = 100  # steps buffered per output chunk

LAST_EXEC_NS = None
_CACHED_NC = None


def _split_excess_waits(nc, limit=1):
    """The walrus build in this container rejects instructions carrying more
    than one sem wait; hoist extra waits onto same-engine NoOps."""
    import bass_rust
    for f in nc.m.functions:
        for bb in f.blocks:
            new_insts = []
            for ins in bb.instructions:
                si = ins.sync_info
                if si is not None and si.on_wait and len(si.on_wait) > limit:
                    waits = list(si.on_wait)
                    head, tail = waits[:-limit], waits[-limit:]
                    for j, w in enumerate(head):
                        c = bass_rust.InstNoOp(name=f"{ins.name}-w{j}")
                        c.engine = ins.engine
                        c.sync_info = mybir.SyncInfo(on_wait=[w], on_update=[])
                        new_insts.append(c)
                    si.on_wait = tail
                new_insts.append(ins)
            bb.instructions = new_insts
    return nc


def _build_nc(n=N, t_total=T, tc_steps=TC, n_in=N_IN, bc=B // N_CORES):
    G = n // 128
    n_chunks = t_total // tc_steps
    NB = (n + 511) // 512
    nb_sizes = [min(512, n - 512 * i) for i in range(NB)]

    assert NB == 2 and G % 2 == 0
    Gh = G // 2  # g-tiles per n-half

    nc = bass.Bass()
    sel_dram = nc.dram_tensor("sel", [128, 8], FP16, kind="ExternalInput")
    w_dram = nc.dram_tensor("w", [128, G * n], FP16, kind="ExternalInput")
    win_dram = nc.dram_tensor("win", [n_in, n], FP16, kind="ExternalInput")
    inpT_dram = nc.dram_tensor("inpT", [n_in, t_total, bc], FP16,
                               kind="ExternalInput")
    x_dram = nc.dram_tensor("xout", [bc, n, t_total], FP32,
                            kind="ExternalOutput")
    x_dram_r = x_dram.rearrange("b (g p) t -> p g b t", p=128)

    with tile.TileContext(nc) as tc:
        with (
            tc.tile_pool(name="const", bufs=1) as const_pool,
            tc.tile_pool(name="state", bufs=1) as state_pool,
            tc.tile_pool(name="work", bufs=3) as work_pool,
            tc.tile_pool(name="obuf", bufs=2) as obuf_pool,
            tc.tile_pool(name="inp", bufs=2) as inp_pool,
            tc.tile_pool(name="psum", bufs=2, space="PSUM") as psum_pool,
            tc.tile_pool(name="psumS", bufs=1, space="PSUM") as psum_static,
        ):
            w_sb = const_pool.tile([128, G * n], FP16)
            nc.sync.dma_start(w_sb[:, :], w_dram[:, :])
            win_sb = const_pool.tile([n_in, n], FP16)
            nc.sync.dma_start(win_sb[:, :], win_dram[:, :])
            sel_sb = const_pool.tile([128, 8], FP16)
            nc.sync.dma_start(sel_sb[:, :], sel_dram[:, :])
            zero16 = const_pool.tile([128, 512], FP16)
            nc.vector.memset(zero16[:, :], 0.0)

            # 4 static psum banks for the col-tiled z partials (ping-pong per
            # step); zero-filled once so never-written partition rows stay
            # finite (sel rows are 0 there, and PE treats 0*garbage as NaN
            # if garbage were NaN)
            zpsS = [[psum_static.tile([128, 512], FP32, name=f"zps_{h_}_{b_}")
                     for b_ in range(2)] for h_ in range(2)]
            for h_ in range(2):
                for b_ in range(2):
                    nc.tensor.matmul(
                        zpsS[h_][b_][:, :], zero16[:, 0:128], zero16[:, :],
                        start=True, stop=True, skip_group_check=True)

            # State kept three ways, split per n-half for fine dep granularity:
            #   s16[.]  fp16 unscaled sum s_t = x_t + h_t -> matmul operand
            #           (the 0.5 leak is folded into W host-side)
            #   xT[.]   fp32 master of x_{t+1} = 0.5 s_t (exact output)
            #   xh16[.] fp16 of x_t, feeds the critical add s16 = xh16 + h
            s16s = [[state_pool.tile([128, Gh * 8], FP16, name=f"s16_{b_}_{h_}")
                     for h_ in range(2)] for b_ in range(2)]
            xTs = [[state_pool.tile([128, Gh * 8], FP32, name=f"xT{b_}_{h_}")
                    for h_ in range(2)] for b_ in range(2)]
            xh16s = [[state_pool.tile([128, Gh * 8], FP16,
                                      name=f"xh16_{b_}_{h_}")
                      for h_ in range(2)] for b_ in range(2)]
            for b_ in range(2):
                for h_ in range(2):
                    nc.vector.memset(s16s[b_][h_][:, :], 0.0)
                    nc.vector.memset(xTs[b_][h_][:, :], 0.0)
                    nc.vector.memset(xh16s[b_][h_][:, :], 0.0)

            def chunk_body(ci):
                inp_sb = inp_pool.tile([n_in, tc_steps * bc], FP16)
                nc.sync.dma_start(
                    inp_sb[:, :], inpT_dram[:, ds(ci * tc_steps, tc_steps), :])
                obuf = obuf_pool.tile([128, G * 8 * tc_steps], FP32)
                obuf_r = obuf[:, :].rearrange(
                    "p (g b t) -> p g b t", g=G, b=8, t=tc_steps)

                def emit_u(t):
                    # u for step t opens the (zero-initialized) static psum
                    # banks; col-tiled z partials land on top
                    zp = [zpsS[h][t % 2] for h in range(2)]
                    for h in range(2):
                        nc.tensor.matmul(
                            zp[h][0:8, :],
                            inp_sb[:, t * bc: (t + 1) * bc],
                            win_sb[:, 512 * h: 512 * (h + 1)],
                            start=True, stop=False, skip_group_check=True,
                        )
                    return zp

                zps_cur = emit_u(0)
                for t in range(tc_steps):
                    xT, xT_n = xTs[t % 2], xTs[(t + 1) % 2]
                    xh16, xh16_n = xh16s[t % 2], xh16s[(t + 1) % 2]
                    s16, s16_n = s16s[t % 2], s16s[(t + 1) % 2]
                    zps = zps_cur
                    for h in range(2):
                        nsl = slice(512 * h, 512 * (h + 1))
                        for g in range(G):
                            nc.tensor.matmul(
                                zps[h][0:8, :],
                                s16[g // Gh][:, (g % Gh) * 8: (g % Gh) * 8 + 8],
                                w_sb[:, g * n + nsl.start: g * n + nsl.stop],
                                start=False, stop=(g == G - 1),
                                skip_group_check=True,
                            )
                    if t + 1 < tc_steps:
                        zps_cur = emit_u(t + 1)  # fills the PE tail gap
                    for h in range(2):
                        # strip-reduce + transpose fused on PE:
                        # zT[nloc, b] = sum_p zp16[p, nloc] * sel[p, b]
                        zp16 = work_pool.tile([128, 512], FP16, tag=f"zp{h}",
                                              name=f"zp{h}")
                        nc.scalar.copy(zp16[:, :], zps[h][:, :])
                        zTp = psum_pool.tile([128, Gh * 8], FP32,
                                             tag=f"zT{h}", name=f"zTp{h}")
                        for c in range(4):
                            nc.tensor.matmul(
                                zTp[:, 8 * c: 8 * c + 8],
                                zp16[:, 128 * c: 128 * c + 128],
                                sel_sb[:, :],
                                start=(c == 0), stop=(c == 3),
                                skip_group_check=True,
                            )
                        hT = work_pool.tile([128, Gh * 8], FP32, tag=f"hT{h}",
                                            name=f"hT{h}")
                        nc.scalar.activation(
                            hT[:, :], zTp[:, :],
                            mybir.ActivationFunctionType.Tanh)
                        # critical: next matmul operand in one add
                        nc.vector.tensor_add(
                            s16_n[h][:, :], xh16[h][:, :], hT[:, :])
                        # off the critical path: fp32 master + output
                        s = work_pool.tile([128, Gh * 8], FP32, tag=f"s{h}",
                                           name=f"s{h}")
                        s_r = s[:, :].rearrange("p (g b) -> p g b", g=Gh, b=8)
                        nc.vector.tensor_add(s[:, :], xT[h][:, :], hT[:, :])
                        nc.scalar.mul(xT_n[h][:, :], s[:, :], ALPHA)
                        nc.vector.tensor_scalar_mul(
                            xh16_n[h][:, :], s[:, :], ALPHA)
                        nc.scalar.mul(
                            obuf_r[:, Gh * h: Gh * (h + 1), :, t],
                            s_r[:, :, :], ALPHA)

                for g in range(G):
                    nc.sync.dma_start(
                        x_dram_r[:, g, :, ds(ci * tc_steps, tc_steps)],
                        obuf_r[:, g, :, :],
                    )

            with tc.For_i(0, n_chunks, 1) as i:
                chunk_body(i)

    _split_excess_waits(nc)
    return nc


def kernel(Input, W_in, W):
    """Full inputs in, full output out. Shards batch over 8 NeuronCores."""
    global LAST_EXEC_NS, _CACHED_NC
    Input = np.ascontiguousarray(np.asarray(Input, dtype=np.float32))
    W_in = np.ascontiguousarray(np.asarray(W_in, dtype=np.float32))
    W = np.ascontiguousarray(np.asarray(W, dtype=np.float32))
    Bf, n_in, t_total = Input.shape
    n = W.shape[0]
    G = n // 128
    bc = Bf // N_CORES

    tc_steps = TC if t_total % TC == 0 else max(
        d for d in range(1, min(TC, t_total) + 1) if t_total % d == 0)
    if _CACHED_NC is None:
        _CACHED_NC = _build_nc(n=n, t_total=t_total, tc_steps=tc_steps,
                               n_in=n_in, bc=bc)
    nc = _CACHED_NC

    # leak factor folded into W: matmul operand is s = x + h = 2x, so W/2
    w_r = np.ascontiguousarray(
        (ALPHA * W).reshape(G, 128, n).transpose(1, 0, 2).reshape(128, G * n)
    ).astype(np.float16)
    win16 = W_in.astype(np.float16)
    sel = np.zeros((128, 8), dtype=np.float16)
    for b_ in range(8):
        sel[b_, b_] = 1.0
    in_maps = []
    for c in range(N_CORES):
        inpT = np.ascontiguousarray(
            Input[c * bc:(c + 1) * bc].transpose(1, 2, 0)).astype(np.float16)
        in_maps.append({"w": w_r, "win": win16, "inpT": inpT, "sel": sel})

    trace = bool(int(os.environ.get("ESN_TRACE", "0")))
    res = run_bass_kernel_spmd(
        nc, in_maps, core_ids=list(range(N_CORES)), trace=trace)
    LAST_EXEC_NS = res.exec_time_ns

    out = np.concatenate([res.results[c]["xout"] for c in range(N_CORES)],
                         axis=0)
    return np.ascontiguousarray(out.astype(np.float32))



# revision 2
# speedup vs baseline: 1.1388x; 1.1388x over previous
"""Trainium2 Bass kernel v2 for the ESN forward scan.

  x_{t+1} = (1-a) x_t + a tanh(u_t + x_t @ W),  a = 0.5
  U = einsum('bit,in->tbn', Input, W_in);  out X[b,n,t] = x_{t+1}[b,n]

Sharding: data-parallel over batch (B=64 -> 8 cores x 8 batches), W/W_in
replicated, no collectives.

v2 structural changes vs baseline:
- Main matmul uses tile_position col-tiling: G_CT concurrent col-groups,
  each computing FULL-K z for a different 128-col n-strip (partition slice
  32j holds strip s=p*G_CT+j). Cuts the W moving-stream ~G_CT x.
- No cross-group reduction needed; the sel matmul is a pure transpose
  (one [128, 8*G_CT]-wide MM per phase).
- has_written priming: one full-width zero-MM (start=True) per phase bank,
  then all main MMs run flags=0 and accumulate per-element.
- U precomputed per chunk as U.T in SBUF ([128 n-part, (s,t,b)] fp16) with
  moving=inp (64 cyc/step amortized), added on DVE before tanh.
- fp16-only state s_t = x_t + h_t (leak 0.5 folded into W host-side);
  x_{t+1} = 0.5*s_t produced as fp16 and cast to fp32 by the gpsimd
  (SWDGE) output DMA.
"""

import os
import numpy as np

import concourse.bass as bass
import concourse.mybir as mybir
import concourse.tile as tile
from concourse.bass import ds
from concourse.bass_utils import run_bass_kernel_spmd

FP32 = mybir.dt.float32
FP16 = mybir.dt.float16
ALU = mybir.AluOpType

ALPHA = 0.5
N_CORES = 8
B, N_IN, T, N = 64, 16, 2000, 1024
TC = 100           # steps per output chunk
G_CT = int(os.environ.get("ESN_GROUPS", "4"))   # concurrent col-groups
NOGPS_DMA = bool(int(os.environ.get("V_NOGPS_DMA", "1")))  # fp32 obuf + SP dma
NOGPS_MUL = bool(int(os.environ.get("V_NOGPS_MUL", "0")))  # obuf mul on DVE
N_DUMMY = int(os.environ.get("ESN_DUMMIES", "5"))  # HAM warm-keeper MMs/step
DUMMY_W = int(os.environ.get("ESN_DUMMY_W", "256"))

LAST_EXEC_NS = None
_CACHED_NC = None


def _split_excess_waits(nc, limit=1):
    """This walrus build rejects instructions carrying more than one sem
    wait; hoist extra waits onto same-engine NoOps."""
    import bass_rust
    for f in nc.m.functions:
        for bb in f.blocks:
            new_insts = []
            for ins in bb.instructions:
                si = ins.sync_info
                if si is not None and si.on_wait and len(si.on_wait) > limit:
                    waits = list(si.on_wait)
                    head, tail = waits[:-limit], waits[-limit:]
                    for j, w in enumerate(head):
                        c = bass_rust.InstNoOp(name=f"{ins.name}-w{j}")
                        c.engine = ins.engine
                        c.sync_info = mybir.SyncInfo(on_wait=[w], on_update=[])
                        new_insts.append(c)
                    si.on_wait = tail
                new_insts.append(ins)
            bb.instructions = new_insts
    return nc


def _build_nc(n=N, t_total=T, tc_steps=TC, n_in=N_IN, bc=B // N_CORES):
    S = n // 128                  # n-strips (8)
    F = S // G_CT                 # phases per step
    n_chunks = t_total // tc_steps
    SW = 8 * G_CT                 # sel width / per-phase state cols

    nc = bass.Bass()
    sel_dram = nc.dram_tensor("sel", [128, SW], FP16, kind="ExternalInput")
    w_dram = nc.dram_tensor("w", [128, S * n], FP16, kind="ExternalInput")
    win_dram = nc.dram_tensor("win", [n_in, n], FP16, kind="ExternalInput")
    inpT_dram = nc.dram_tensor("inpT", [n_in, t_total, bc], FP16,
                               kind="ExternalInput")
    x_dram = nc.dram_tensor("xout", [bc, n, t_total], FP32,
                            kind="ExternalOutput")
    x_dram_r = x_dram.rearrange("b (s p) t -> p s b t", p=128)

    with tile.TileContext(nc) as tc:
        with (
            tc.tile_pool(name="const", bufs=1) as const_pool,
            tc.tile_pool(name="state", bufs=1) as state_pool,
            tc.tile_pool(name="work", bufs=3) as work_pool,
            tc.tile_pool(name="obuf", bufs=2) as obuf_pool,
            tc.tile_pool(name="inp", bufs=2) as inp_pool,
            tc.tile_pool(name="upsum", bufs=2, space="PSUM") as upsum_pool,
            tc.tile_pool(name="psumS", bufs=1, space="PSUM") as psum_static,
        ):
            w_sb = const_pool.tile([128, S * n], FP16)
            nc.sync.dma_start(w_sb[:, :], w_dram[:, :])
            win_sb = const_pool.tile([n_in, n], FP16)
            nc.sync.dma_start(win_sb[:, :], win_dram[:, :])
            sel_sb = const_pool.tile([128, SW], FP16)
            nc.sync.dma_start(sel_sb[:, :], sel_dram[:, :])
            zero16 = const_pool.tile([128, 128], FP16)
            nc.vector.memset(zero16[:, :], 0.0)

            # UT buffer: U.T for the chunk, [128 n_lo, (s, t, b)] fp16
            ut_sb = const_pool.tile([128, S * tc_steps * bc], FP16)
            ut_r = ut_sb[:, :].rearrange("p (s t b) -> p s t b",
                                         s=S, t=tc_steps, b=bc)

            # psum banks: B (z per phase, ping-pong) and Z (zT per phase,
            # ping-pong). Allocate full banks to guarantee no sharing.
            zb = [psum_static.tile([128, 512], FP32, name=f"zb{i}")
                  for i in range(2)]
            zt = [psum_static.tile([128, 512], FP32, name=f"zt{i}")
                  for i in range(2)]
            scratch = psum_static.tile([128, 512], FP32, name="scratch")
            # prime zt has_written bits once; per-step sel accumulates onto
            # DVE-preloaded uT content (engine writes don't touch the bits)
            for i in range(2):
                nc.tensor.matmul(zt[i][0:128, 0:SW], zero16[:, :],
                                 zero16[:, 0:SW], start=True, stop=True,
                                 skip_group_check=True)

            # state s16 (parity ping-pong): [128 n_lo, (s, b)] fp16
            s16s = [state_pool.tile([128, S * bc], FP16, name=f"s16_{i}")
                    for i in range(2)]
            for i in range(2):
                nc.vector.memset(s16s[i][:, :], 0.0)

            def chunk_body(ci):
                inp_sb = inp_pool.tile([n_in, tc_steps * bc], FP16)
                nc.sync.dma_start(
                    inp_sb[:, :], inpT_dram[:, ds(ci * tc_steps, tc_steps), :])
                obuf = obuf_pool.tile([128, S * bc * tc_steps],
                                      FP32 if NOGPS_DMA else FP16)
                obuf_r = obuf[:, :].rearrange(
                    "p (s b t) -> p s b t", s=S, b=bc, t=tc_steps)

                # ---- U.T precompute: per strip, 2 t-halves --------------
                th = tc_steps // 2
                for s in range(S):
                    for h in range(2):
                        ups = upsum_pool.tile([128, th * bc], FP32,
                                              tag="ups", name=f"ups{s}_{h}")
                        nc.tensor.matmul(
                            ups[:, :],
                            win_sb[:, 128 * s: 128 * (s + 1)],
                            inp_sb[:, h * th * bc: (h + 1) * th * bc],
                            start=True, stop=True, skip_group_check=True)
                        dst = ut_r[:, s, h * th: (h + 1) * th, :] \
                            .rearrange("p t b -> p (t b)")
                        if (s + h) % 2 == 0:
                            nc.vector.tensor_copy(dst, ups[:, :])
                        else:
                            nc.scalar.copy(dst, ups[:, :])

                # ---- the scan -------------------------------------------
                def dummy_mm():
                    # HAM warm-keeper: scratch matmul, no consumers
                    nc.tensor.matmul(
                        scratch[0:bc, 0:DUMMY_W], sel_sb[:, 0:bc],
                        w_sb[:, 0:DUMMY_W], start=True, stop=True,
                        skip_group_check=True)

                for t in range(tc_steps):
                    s16, s16_n = s16s[t % 2], s16s[(t + 1) % 2]
                    # DVE: preload uT(t) content into zt banks (off-chain)
                    for p in range(F):
                        nc.vector.tensor_copy(
                            zt[p % 2][0:128, 0:SW].rearrange(
                                "q (g b) -> q g b", g=G_CT),
                            ut_r[:, p * G_CT: (p + 1) * G_CT, t, :])
                    # phase mains (zero-MM + G_CT col-groups x S k-chunks)
                    for p in range(F):
                        bank = zb[p % 2]
                        nc.tensor.matmul(
                            bank[:, 0:128], zero16[:, :], zero16[:, :],
                            start=True, stop=False, skip_group_check=True)
                        for k in range(S):
                            for j in range(G_CT):
                                strip = p * G_CT + j
                                nc.tensor.matmul(
                                    bank[32 * j: 32 * j + bc, 0:128],
                                    s16[:, bc * k: bc * (k + 1)],
                                    w_sb[:, k * n + 128 * strip:
                                         k * n + 128 * (strip + 1)],
                                    start=False,
                                    stop=(k == S - 1 and j == G_CT - 1),
                                    skip_group_check=True,
                                    tile_position=(0, 32 * j),
                                )
                    # psum -> sbuf copies: p0 on ACT, p1 on DVE
                    zp16s = []
                    for p in range(F):
                        zp16 = work_pool.tile([128, 128], FP16,
                                              tag=f"zp{p}", name=f"zp{p}")
                        if p % 2 == 0:
                            nc.scalar.copy(zp16[:, :], zb[p % 2][:, 0:128])
                        else:
                            nc.vector.tensor_copy(zp16[:, :],
                                                  zb[p % 2][:, 0:128])
                        zp16s.append(zp16)
                    # sel transposes (accumulate onto preloaded uT) + warmers
                    for p in range(F):
                        nc.tensor.matmul(
                            zt[p % 2][0:128, 0:SW], zp16s[p][:, :],
                            sel_sb[:, :],
                            start=False, stop=True, skip_group_check=True)
                        nd = (N_DUMMY + 1) // 2 if p == 0 else N_DUMMY // 2
                        for _ in range(nd):
                            dummy_mm()
                    # tails
                    for p in range(F):
                        hT = work_pool.tile([128, SW], FP16,
                                            tag=f"hT{p}", name=f"hT{p}")
                        nc.scalar.activation(
                            hT[:, :], zt[p % 2][0:128, 0:SW],
                            mybir.ActivationFunctionType.Tanh)
                        # s' = 0.5*s + h   (fp16, fused)
                        nc.vector.scalar_tensor_tensor(
                            s16_n[:, SW * p: SW * (p + 1)],
                            s16[:, SW * p: SW * (p + 1)],
                            0.5, hT[:, :],
                            op0=ALU.mult, op1=ALU.add)
                        # x_{t+1} = 0.5*s' -> output buffer
                        oeng = nc.vector if NOGPS_MUL else nc.gpsimd
                        oeng.tensor_scalar_mul(
                            obuf_r[:, p * G_CT: (p + 1) * G_CT, :, t],
                            s16_n[:, SW * p: SW * (p + 1)]
                            .rearrange("q (g b) -> q g b", g=G_CT),
                            0.5)

                for s in range(S):
                    deng = nc.sync if NOGPS_DMA else nc.gpsimd
                    deng.dma_start(
                        x_dram_r[:, s, :, ds(ci * tc_steps, tc_steps)],
                        obuf_r[:, s, :, :],
                    )

            with tc.For_i(0, n_chunks, 1) as i:
                chunk_body(i)

    _split_excess_waits(nc)
    return nc


def kernel(Input, W_in, W):
    """Full inputs in, full output out. Shards batch over 8 NeuronCores."""
    global LAST_EXEC_NS, _CACHED_NC
    Input = np.ascontiguousarray(np.asarray(Input, dtype=np.float32))
    W_in = np.ascontiguousarray(np.asarray(W_in, dtype=np.float32))
    W = np.ascontiguousarray(np.asarray(W, dtype=np.float32))
    Bf, n_in, t_total = Input.shape
    n = W.shape[0]
    S = n // 128
    bc = Bf // N_CORES

    tc_steps = TC if t_total % TC == 0 else max(
        d for d in range(1, min(TC, t_total) + 1) if t_total % d == 0)
    if _CACHED_NC is None:
        _CACHED_NC = _build_nc(n=n, t_total=t_total, tc_steps=tc_steps,
                               n_in=n_in, bc=bc)
    nc = _CACHED_NC

    # leak folded into W: matmul operand is s = x + h = 2x, so W/2.
    # w layout: w_r[p, k*n + m] = (ALPHA*W)[128*k + p, m]
    w_r = np.ascontiguousarray(
        (ALPHA * W).reshape(S, 128, n).transpose(1, 0, 2).reshape(128, S * n)
    ).astype(np.float16)
    win16 = W_in.astype(np.float16)
    SW = 8 * G_CT
    sel = np.zeros((128, SW), dtype=np.float16)
    for j in range(G_CT):
        for b_ in range(bc):
            sel[32 * j + b_, 8 * j + b_] = 1.0
    in_maps = []
    for c in range(N_CORES):
        inpT = np.ascontiguousarray(
            Input[c * bc:(c + 1) * bc].transpose(1, 2, 0)).astype(np.float16)
        in_maps.append({"w": w_r, "win": win16, "inpT": inpT, "sel": sel})

    trace = bool(int(os.environ.get("ESN_TRACE", "0")))
    res = run_bass_kernel_spmd(
        nc, in_maps, core_ids=list(range(N_CORES)), trace=trace)
    LAST_EXEC_NS = res.exec_time_ns

    out = np.concatenate([res.results[c]["xout"] for c in range(N_CORES)],
                         axis=0)
    return np.ascontiguousarray(out.astype(np.float32))
